# revision 14
# baseline (speedup 1.0000x reference)
"""KAN layer kernel for TRN2, 8-core SPMD.

Math: out[b,o] = sum_{i,k} relu(x[b,i]*w1[o,i,k] + b1[o,i,k]) * w2[o,i,k] / 32 + b2[o]
With b1 == 0 (guaranteed by the generator) the relu is exactly
    relu(x*w1) = (x*w1 + |x|*|w1|) / 2
so the layer collapses to two matmuls with k-pre-reduced weights:
    T[i,o] = sum_k w1*w2        U[i,o] = sum_k |w1|*w2
    out    = x @ T/64 + |x| @ U/64 + b2          (1/64 = 0.5/32)
The /64 is folded into the staged w1 (exact: a power of two), so the
epilogue is a pure PSUM->bf16 copy and the T-term matmul rhs is the raw
DMA'd x (no min/relu work on x at all; only |x| for the U term).

Sharding: 4 batch groups x 2 dout groups (core = bi*2 + oj); per core
BS=512 batch rows, OS=128 dout cols, 2 din tiles of 128.

Schedule (cost-model ns):  transfers serialize on the DMA engines at
~364ns/128KB; descriptor gen pipelines on SP-HWDGE (625/dma) and Pool
SWDGE (~1040/dma, parallel path).  ACT issues no DMA - an ACT DMA would
steal the 2nd HWDGE slot from SP.  Tile-0's weights arrive as two
k-pair chunks so DVE prep starts at ~3.2us instead of ~3.6:
  SP  : w-t0-kp0 | w-t1 | x-t1h0 | out
  Pool: w-t0-kp1 | x-t0 | x-t1h1 | |x1h1| (abs)
  ACT : |x0|, |x1h0|, epilogue-h0
  DVE : per w chunk: |w1| (tensor_scalar abs_max, 4x mode), cat =
        [w1,|w1|] (x) w2-broadcast (tensor_tensor, 2x), k-sums as
        pair-adds; epilogue-h1
  PE  : warm-up (p-state ramp), then T0/U0/T1/U1 accumulating matmuls
        per psum half (asymmetric 320/192 split so the tail half is
        cheap).
"""

import numpy as np

B, DIN, DOUT, K = 2048, 256, 256, 4
N_CORES = 8
BG, OG = 4, 2                      # batch groups x dout groups
BS, OS = B // BG, DOUT // OG       # 512 batch rows, 128 dout cols per core
NT = DIN // 128                    # din tiles
HB = BS // 2                       # batch half (x dma granularity)
HB0, HB1 = 320, 192                # asymmetric psum halves
WROW = 2 * OS * K                  # 1024 bf16 els per (tile, din) row
N_WARM = 45                        # PE warm-up matmuls ([128,128] each)
N_WARM2 = 2                        # mid warm-ups between U0 and T1 matmuls

_CACHE = {}


def _build_nc():
    if "nc" in _CACHE:
        return _CACHE["nc"]
    import concourse.bacc as bacc
    import concourse.tile as tile
    from concourse import mybir

    f32 = mybir.dt.float32
    bf16 = mybir.dt.bfloat16
    AF = mybir.ActivationFunctionType
    OP = mybir.AluOpType
    X = mybir.AxisListType.X

    nc = bacc.Bacc("TRN2", target_bir_lowering=False, debug=False,
                   num_devices=N_CORES, num_swdge_queues=2)
    # t0 rows: [kp, 2(which), OS, 2] (k-pair major); t1 rows: [2, OS, K]
    wt = nc.dram_tensor("wt", [NT * 128, WROW], bf16, kind="ExternalInput")
    # x rows: [(t*128+i)*2 + h] -> 256 batch cols (h = batch half)
    xt = nc.dram_tensor("xt", [NT * 128 * 2, HB], bf16, kind="ExternalInput")
    outt = nc.dram_tensor("outt", [OS, BS], bf16, kind="ExternalOutput")

    with tile.TileContext(nc) as tc:
        with (
            tc.tile_pool(name="io", bufs=1) as io,
            tc.tile_pool(name="work", bufs=1) as work,
            tc.tile_pool(name="pp", bufs=1, space="PSUM") as pp,
        ):
            # ---- SBUF tiles.  w chunk tiles have 3 slots: 0=w1 (dma),
            # 1=|w1| (computed), 2=w2 (dma); the cat-mul reads [:, 0::2].
            wa = io.tile([128, 3, 2, OS], bf16, name="wa", tag="wa")
            wb = io.tile([128, 3, 2, OS], bf16, name="wb", tag="wb")
            w1t = io.tile([128, 3, K, OS], bf16, name="w1t", tag="w1t")
            x_sb = [io.tile([128, BS], bf16, name=f"x{t}", tag=f"x{t}")
                    for t in range(NT)]
            xa_sb = [io.tile([128, BS], bf16, name=f"xa{t}", tag=f"xa{t}")
                     for t in range(NT)]
            out_sb = work.tile([128, BS], bf16, tag="outsb")
            zt = work.tile([128, 128], bf16, tag="zt")

            # ---- input DMAs
            # SP (HWDGE): w-t0-kp0, w-t1, x-t1h0
            nc.sync.dma_start(out=wa[:, 0::2], in_=wt[0:128, 0:512])
            nc.sync.dma_start(out=w1t[:, 0::2], in_=wt[128:256, :])
            nc.sync.dma_start(out=x_sb[1][:, 0:HB], in_=xt[256:512:2, :])
            # Pool (SWDGE): w-t0-kp1, x-t0, x-t1h1
            nc.gpsimd.dma_start(out=wb[:, 0::2], in_=wt[0:128, 512:1024])
            nc.gpsimd.dma_start(out=x_sb[0], in_=xt[0:256, :])
            nc.gpsimd.dma_start(out=x_sb[1][:, HB:BS], in_=xt[257:512:2, :])

            # ---- PE warm-up (p-state ramp; matmuls hit 2.4GHz only after
            # ~3us of PE activity in the cost model)
            nc.vector.memset(zt, 0.0)
            pz = pp.tile([128, 128], f32, tag="pz")
            for _ in range(N_WARM):
                nc.tensor.matmul(pz, lhsT=zt, rhs=zt, start=True, stop=True)

            # ---- DVE weight prep
            # per chunk: |w1| -> cat=[w1,|w1|]*w2 -> pair-sum over k
            s0 = work.tile([128, 2, 2, OS], bf16, name="s0", tag="s0")
            st0 = work.tile([128, 2, OS], bf16, name="st0", tag="st0")
            st1 = work.tile([128, 2, OS], bf16, name="st1", tag="st1")
            su0 = work.tile([128, OS], bf16, name="su0", tag="su0")
            su1 = work.tile([128, OS], bf16, name="su1", tag="su1")
            # wait_until hints (~data-landing times) keep the list scheduler
            # from hoisting chunk-b / tile-1 ops in front of ready chunk-a
            # work, which would idle DVE.
            with nc.allow_low_precision(reason="bf16 k-sum, tol 2e-2"):
                for c, wc in enumerate((wa, wb)):
                    cat = work.tile([128, 2, 2, OS], bf16, name=f"cat{c}",
                                    tag=f"cat{c}")
                    with tc.tile_wait_until(0.0 if c == 0 else 0.0037):
                        nc.vector.tensor_scalar(out=wc[:, 1], in0=wc[:, 0],
                                                scalar1=0.0, scalar2=2.0,
                                                op0=OP.max, op1=OP.mult)
                        nc.vector.tensor_tensor(
                            out=cat, in0=wc[:, 0:2],
                            in1=wc[:, 2].unsqueeze(1).broadcast_to(
                                [128, 2, 2, OS]),
                            op=OP.mult)
                        nc.vector.tensor_tensor(out=s0[:, c],
                                                in0=cat[:, :, 0],
                                                in1=cat[:, :, 1], op=OP.add)
                # st0 + subU0 on Pool: frees the serial DVE chain; Pool
                # is idle once its DMA descriptor-gens are done.
                nc.gpsimd.tensor_tensor(out=st0, in0=s0[:, 0], in1=s0[:, 1],
                                        op=OP.add)
                nc.gpsimd.tensor_tensor(out=su0, in0=st0[:, 1],
                                        in1=st0[:, 0], op=OP.subtract)
                # tile 1 in one piece: abs, cat-mul, k pair-sums, final sum
                cat1 = work.tile([128, 2, K, OS], bf16, name="cat1", tag="cat1")
                s21 = work.tile([128, 2, 2, OS], bf16, name="s21", tag="s21")
                with tc.tile_wait_until(0.0045):
                    nc.vector.tensor_scalar(out=w1t[:, 1], in0=w1t[:, 0],
                                            scalar1=0.0, scalar2=2.0,
                                            op0=OP.max, op1=OP.mult)
                    nc.vector.tensor_tensor(
                        out=cat1, in0=w1t[:, 0:2],
                        in1=w1t[:, 2].unsqueeze(1).broadcast_to(
                            [128, 2, K, OS]),
                        op=OP.mult)
                    nc.vector.tensor_tensor(out=s21, in0=cat1[:, :, 0:2],
                                            in1=cat1[:, :, 2:4], op=OP.add)
                    nc.vector.tensor_tensor(out=st1, in0=s21[:, :, 0],
                                            in1=s21[:, :, 1], op=OP.add)
                    nc.vector.tensor_tensor(out=su1, in0=st1[:, 1],
                                            in1=st1[:, 0], op=OP.subtract)

            # ---- |x| on ACT (x1 split so the h0 strip lands first)
            nc.scalar.activation(xa_sb[0], x_sb[0], AF.Abs)
            nc.scalar.activation(xa_sb[1][:, 0:HB0], x_sb[1][:, 0:HB0], AF.Abs)
            nc.scalar.activation(xa_sb[1][:, HB0:BS], x_sb[1][:, HB0:BS],
                                 AF.Abs)

            # ---- matmuls: psum halves = batch slices [0:320) [320:512)
            s_t = (st0, st1)
            psum = [pp.tile([128, hb], f32, name=f"ps{h}", tag=f"ps{h}")
                    for h, hb in enumerate((HB0, HB1))]
            sl = (slice(0, HB0), slice(HB0, BS))
            MM = [
                (0, 0, 0, True, False), (0, 0, 1, True, False),   # T0 h0,h1
                (0, 1, 0, False, False), (0, 1, 1, False, False), # U0
                (1, 0, 0, False, False),                          # T1 h0
                (1, 1, 0, False, True),                           # U1 h0 stop
                (1, 0, 1, False, False),                          # T1 h1
                (1, 1, 1, False, True),                           # U1 h1 stop
            ]
            su = (su0, su1)
            for i, (t, j, h, st, sp) in enumerate(MM):
                if i == 4:
                    for _ in range(N_WARM2):
                        nc.tensor.matmul(pz, lhsT=zt, rhs=zt,
                                         start=True, stop=True)
                rhs = (x_sb if j == 0 else xa_sb)[t][:, sl[h]]
                lhsT = s_t[t][:, 0] if j == 0 else su[t]
                nc.tensor.matmul(psum[h], lhsT=lhsT, rhs=rhs,
                                 start=st, stop=sp)

            # ---- epilogue: pure copy psum -> bf16 (scale folded into w1s)
            EH = HB0 - 128
            nc.scalar.activation(out_sb[:, 0:EH], psum[0][:, 0:EH], AF.Identity)
            nc.gpsimd.tensor_copy(out_sb[:, EH:HB0], psum[0][:, EH:HB0])
            nc.vector.tensor_copy(out_sb[:, HB0:BS], psum[1])

            # ---- output
            nc.sync.dma_start(out=outt[:, :], in_=out_sb)

    nc.compile()
    _CACHE["nc"] = nc
    return nc


def _kan_numpy(x, w1, b1, w2, b2):
    out = np.empty((x.shape[0], w1.shape[0]), dtype=np.float32)
    d = (w1.shape[0] + w1.shape[1]) / 2
    s = 1.0 / np.sqrt(d * w1.shape[2])
    for lo in range(0, x.shape[0], 128):
        hi = min(lo + 128, x.shape[0])
        h = x[lo:hi, None, :, None] * w1[None] + b1[None]
        np.maximum(h, 0.0, out=h)
        out[lo:hi] = np.einsum("boik,oik->bo", h, w2) * s
    return out + b2[None, :]


def kernel(x, w1, b1, w2, b2):
    x = np.asarray(x, dtype=np.float32)
    w1 = np.asarray(w1, dtype=np.float32)
    b1 = np.asarray(b1, dtype=np.float32)
    w2 = np.asarray(w2, dtype=np.float32)
    b2 = np.asarray(b2, dtype=np.float32)

    if (x.shape != (B, DIN) or w1.shape != (DOUT, DIN, K)
            or np.any(b1) or np.any(b2)):
        return _kan_numpy(x, w1, b1, w2, b2)

    import ml_dtypes
    from concourse.bass_utils import run_bass_kernel_spmd

    nc = _build_nc()
    bf16 = ml_dtypes.bfloat16

    xT = np.ascontiguousarray(x.T).astype(bf16)          # (DIN, B)
    w1k = (w1 / 64.0).transpose(1, 2, 0).astype(bf16)    # (DIN, K, DOUT)
    w2k = w2.transpose(1, 2, 0).astype(bf16)

    in_maps = []
    for core in range(N_CORES):
        bi, oj = divmod(core, OG)
        osl = slice(oj * OS, (oj + 1) * OS)
        bsl = slice(bi * BS, (bi + 1) * BS)
        # tile-0 rows: [kp, which, k2, OS]; tile-1 rows: [which, K, OS]
        w0 = np.empty((128, 2, 2, 2, OS), dtype=bf16)
        for kp in range(2):
            w0[:, kp, 0] = w1k[0:128, 2 * kp:2 * kp + 2, osl]
            w0[:, kp, 1] = w2k[0:128, 2 * kp:2 * kp + 2, osl]
        w1r = np.empty((128, 2, K, OS), dtype=bf16)
        w1r[:, 0] = w1k[128:256, :, osl]
        w1r[:, 1] = w2k[128:256, :, osl]
        wtc = np.empty((NT * 128, WROW), dtype=bf16)
        wtc[0:128] = w0.reshape(128, WROW)
        wtc[128:256] = w1r.reshape(128, WROW)
        xtc = np.ascontiguousarray(xT[:, bsl]).reshape(NT * 128 * 2, HB)
        in_maps.append({"wt": wtc, "xt": xtc})

    res = run_bass_kernel_spmd(nc, in_maps, core_ids=list(range(N_CORES)))

    out = np.empty((B, DOUT), dtype=np.float32)
    for core in range(N_CORES):
        bi, oj = divmod(core, OG)
        out[bi * BS:(bi + 1) * BS, oj * OS:(oj + 1) * OS] = \
            res.results[core]["outt"].astype(np.float32).T
    return out


# revision 15
# speedup vs baseline: 1.0211x; 1.0211x over previous
"""KAN layer kernel for TRN2, 8-core SPMD.

Math: out[b,o] = sum_{i,k} relu(x[b,i]*w1[o,i,k] + b1[o,i,k]) * w2[o,i,k] / 32 + b2[o]
With b1 == 0 (guaranteed by the generator) the relu is exactly
    relu(x*w1) = (x*w1 + |x|*|w1|) / 2
so the layer collapses to two matmuls with k-pre-reduced weights:
    T[i,o] = sum_k w1*w2        U[i,o] = sum_k |w1|*w2
    out    = x @ T/64 + |x| @ U/64 + b2          (1/64 = 0.5/32)
The /64 is folded into the staged w1 (exact: a power of two), so the
epilogue is a pure PSUM->bf16 copy and the T-term matmul rhs is the raw
DMA'd x (no min/relu work on x at all; only |x| for the U term).

Sharding: 4 batch groups x 2 dout groups (core = bi*2 + oj); per core
BS=512 batch rows, OS=128 dout cols, 2 din tiles of 128.

Schedule (cost-model ns):  transfers serialize on the DMA engines at
~364ns/128KB; descriptor gen pipelines on SP-HWDGE (625/dma) and Pool
SWDGE (~1040/dma, parallel path).  ACT issues no DMA - an ACT DMA would
steal the 2nd HWDGE slot from SP.  Tile-0's weights arrive as two
k-pair chunks so DVE prep starts at ~3.2us instead of ~3.6:
  SP  : w-t0-kp0 | w-t1 | x-t1h0 | out
  Pool: w-t0-kp1 | x-t0 | x-t1h1 | |x1h1| (abs)
  ACT : |x0|, |x1h0|, epilogue-h0
  DVE : per w chunk: |w1| (tensor_scalar abs_max, 4x mode), cat =
        [w1,|w1|] (x) w2-broadcast (tensor_tensor, 2x), k-sums as
        pair-adds; epilogue-h1
  PE  : warm-up (p-state ramp), then T0/U0/T1/U1 accumulating matmuls
        per psum half (asymmetric 320/192 split so the tail half is
        cheap).
"""

import numpy as np

B, DIN, DOUT, K = 2048, 256, 256, 4
N_CORES = 8
BG, OG = 4, 2                      # batch groups x dout groups
BS, OS = B // BG, DOUT // OG       # 512 batch rows, 128 dout cols per core
NT = DIN // 128                    # din tiles
HB = BS // 2                       # batch half (x dma granularity)
HB0, HB1 = 320, 192                # asymmetric psum halves
WROW = 2 * OS * K                  # 1024 bf16 els per (tile, din) row
N_WARM = 45                        # PE warm-up matmuls ([128,128] each)
N_WARM2 = 2                        # mid warm-ups between U0 and T1 matmuls

_CACHE = {}


def _build_nc():
    if "nc" in _CACHE:
        return _CACHE["nc"]
    import concourse.bacc as bacc
    import concourse.tile as tile
    from concourse import mybir

    f32 = mybir.dt.float32
    bf16 = mybir.dt.bfloat16
    AF = mybir.ActivationFunctionType
    OP = mybir.AluOpType
    X = mybir.AxisListType.X

    nc = bacc.Bacc("TRN2", target_bir_lowering=False, debug=False,
                   num_devices=N_CORES, num_swdge_queues=2)
    # t0 rows: [kp, 2(which), OS, 2] (k-pair major); t1 rows: [2, OS, K]
    wt = nc.dram_tensor("wt", [NT * 128, WROW], bf16, kind="ExternalInput")
    # x rows: [(t*128+i)*2 + h] -> 256 batch cols (h = batch half)
    xt = nc.dram_tensor("xt", [NT * 128 * 2, HB], bf16, kind="ExternalInput")
    outt = nc.dram_tensor("outt", [OS, BS], bf16, kind="ExternalOutput")

    with tile.TileContext(nc) as tc:
        with (
            tc.tile_pool(name="io", bufs=1) as io,
            tc.tile_pool(name="work", bufs=1) as work,
            tc.tile_pool(name="pp", bufs=1, space="PSUM") as pp,
        ):
            # ---- SBUF tiles.  w chunk tiles have 3 slots: 0=w1 (dma),
            # 1=|w1| (computed), 2=w2 (dma); the cat-mul reads [:, 0::2].
            wa = io.tile([128, 3, 2, OS], bf16, name="wa", tag="wa")
            wb = io.tile([128, 3, 2, OS], bf16, name="wb", tag="wb")
            w1t = io.tile([128, 3, K, OS], bf16, name="w1t", tag="w1t")
            x_sb = [io.tile([128, BS], bf16, name=f"x{t}", tag=f"x{t}")
                    for t in range(NT)]
            xa_sb = [io.tile([128, BS], bf16, name=f"xa{t}", tag=f"xa{t}")
                     for t in range(NT)]
            out_sb = work.tile([128, BS], bf16, tag="outsb")
            zt = work.tile([128, 128], bf16, tag="zt")

            # ---- input DMAs
            # SP (HWDGE): w-t0-kp0, w-t1, x-t1h0
            nc.sync.dma_start(out=wa[:, 0::2], in_=wt[0:128, 0:512])
            nc.sync.dma_start(out=w1t[:, 0::2], in_=wt[128:256, :])
            nc.sync.dma_start(out=x_sb[1][:, 0:HB], in_=xt[256:512:2, :])
            # Pool (SWDGE): w-t0-kp1, x-t0, x-t1h1
            nc.gpsimd.dma_start(out=wb[:, 0::2], in_=wt[0:128, 512:1024])
            nc.gpsimd.dma_start(out=x_sb[0], in_=xt[0:256, :])
            nc.gpsimd.dma_start(out=x_sb[1][:, HB:BS], in_=xt[257:512:2, :])

            # ---- PE warm-up (p-state ramp; matmuls hit 2.4GHz only after
            # ~3us of PE activity in the cost model)
            nc.vector.memset(zt, 0.0)
            pz = pp.tile([128, 128], f32, tag="pz")
            for _ in range(N_WARM):
                nc.tensor.matmul(pz, lhsT=zt, rhs=zt, start=True, stop=True)

            # ---- DVE weight prep
            # per chunk: |w1| -> cat=[w1,|w1|]*w2 -> pair-sum over k
            s0 = work.tile([128, 2, 2, OS], bf16, name="s0", tag="s0")
            st0 = work.tile([128, 2, OS], bf16, name="st0", tag="st0")
            st1 = work.tile([128, 2, OS], bf16, name="st1", tag="st1")
            su0 = work.tile([128, OS], bf16, name="su0", tag="su0")
            su1 = work.tile([128, OS], bf16, name="su1", tag="su1")
            # wait_until hints (~data-landing times) keep the list scheduler
            # from hoisting chunk-b / tile-1 ops in front of ready chunk-a
            # work, which would idle DVE.
            with nc.allow_low_precision(reason="bf16 k-sum, tol 2e-2"):
                for c, wc in enumerate((wa, wb)):
                    cat = work.tile([128, 2, 2, OS], bf16, name=f"cat{c}",
                                    tag=f"cat{c}")
                    with tc.tile_wait_until(0.0 if c == 0 else 0.0037):
                        nc.vector.tensor_scalar(out=wc[:, 1], in0=wc[:, 0],
                                                scalar1=0.0, scalar2=2.0,
                                                op0=OP.max, op1=OP.mult)
                        nc.vector.tensor_tensor(
                            out=cat, in0=wc[:, 0:2],
                            in1=wc[:, 2].unsqueeze(1).broadcast_to(
                                [128, 2, 2, OS]),
                            op=OP.mult)
                        nc.vector.tensor_tensor(out=s0[:, c],
                                                in0=cat[:, :, 0],
                                                in1=cat[:, :, 1], op=OP.add)
                # st0 + subU0 on Pool: frees the serial DVE chain; Pool
                # is idle once its DMA descriptor-gens are done.
                nc.gpsimd.tensor_tensor(out=st0, in0=s0[:, 0], in1=s0[:, 1],
                                        op=OP.add)
                nc.gpsimd.tensor_tensor(out=su0, in0=st0[:, 1],
                                        in1=st0[:, 0], op=OP.subtract)
                # tile 1 in one piece: abs, cat-mul, k pair-sums, final sum
                cat1 = work.tile([128, 2, K, OS], bf16, name="cat1", tag="cat1")
                s21 = work.tile([128, 2, 2, OS], bf16, name="s21", tag="s21")
                with tc.tile_wait_until(0.0045):
                    nc.vector.tensor_scalar(out=w1t[:, 1], in0=w1t[:, 0],
                                            scalar1=0.0, scalar2=2.0,
                                            op0=OP.max, op1=OP.mult)
                    nc.vector.tensor_tensor(
                        out=cat1, in0=w1t[:, 0:2],
                        in1=w1t[:, 2].unsqueeze(1).broadcast_to(
                            [128, 2, K, OS]),
                        op=OP.mult)
                    nc.vector.tensor_tensor(out=s21, in0=cat1[:, :, 0:2],
                                            in1=cat1[:, :, 2:4], op=OP.add)
                    nc.vector.tensor_tensor(out=st1, in0=s21[:, :, 0],
                                            in1=s21[:, :, 1], op=OP.add)
                    nc.vector.tensor_tensor(out=su1, in0=st1[:, 1],
                                            in1=st1[:, 0], op=OP.subtract)

            # ---- |x| on ACT (x1 split so the h0 strip lands first)
            nc.scalar.activation(xa_sb[0], x_sb[0], AF.Abs)
            nc.scalar.activation(xa_sb[1][:, 0:HB0], x_sb[1][:, 0:HB0], AF.Abs)
            nc.scalar.activation(xa_sb[1][:, HB0:BS], x_sb[1][:, HB0:BS],
                                 AF.Abs)

            # ---- matmuls: psum halves = batch slices [0:320) [320:512)
            s_t = (st0, st1)
            psum = [pp.tile([128, hb], f32, name=f"ps{h}", tag=f"ps{h}")
                    for h, hb in enumerate((HB0, HB1))]
            sl = (slice(0, HB0), slice(HB0, BS))
            MM = [
                (0, 0, 0, True, False), (0, 0, 1, True, False),   # T0 h0,h1
                (0, 1, 0, False, False), (0, 1, 1, False, False), # U0
                (1, 0, 0, False, False),                          # T1 h0
                (1, 1, 0, False, True),                           # U1 h0 stop
                (1, 0, 1, False, False),                          # T1 h1
                (1, 1, 1, False, True),                           # U1 h1 stop
            ]
            su = (su0, su1)
            for i, (t, j, h, st, sp) in enumerate(MM):
                if i == 4:
                    for _ in range(N_WARM2):
                        nc.tensor.matmul(pz, lhsT=zt, rhs=zt,
                                         start=True, stop=True)
                rhs = (x_sb if j == 0 else xa_sb)[t][:, sl[h]]
                lhsT = s_t[t][:, 0] if j == 0 else su[t]
                nc.tensor.matmul(psum[h], lhsT=lhsT, rhs=rhs,
                                 start=st, stop=sp)

            # ---- epilogue: pure copy psum -> bf16 (scale folded into w1s)
            nc.scalar.activation(out_sb[:, 0:HB0], psum[0], AF.Identity)
            nc.vector.tensor_copy(out_sb[:, HB0:BS], psum[1])

            # ---- output
            nc.sync.dma_start(out=outt[:, :], in_=out_sb)

    nc.compile()
    _CACHE["nc"] = nc
    return nc


def _kan_numpy(x, w1, b1, w2, b2):
    out = np.empty((x.shape[0], w1.shape[0]), dtype=np.float32)
    d = (w1.shape[0] + w1.shape[1]) / 2
    s = 1.0 / np.sqrt(d * w1.shape[2])
    for lo in range(0, x.shape[0], 128):
        hi = min(lo + 128, x.shape[0])
        h = x[lo:hi, None, :, None] * w1[None] + b1[None]
        np.maximum(h, 0.0, out=h)
        out[lo:hi] = np.einsum("boik,oik->bo", h, w2) * s
    return out + b2[None, :]


def kernel(x, w1, b1, w2, b2):
    x = np.asarray(x, dtype=np.float32)
    w1 = np.asarray(w1, dtype=np.float32)
    b1 = np.asarray(b1, dtype=np.float32)
    w2 = np.asarray(w2, dtype=np.float32)
    b2 = np.asarray(b2, dtype=np.float32)

    if (x.shape != (B, DIN) or w1.shape != (DOUT, DIN, K)
            or np.any(b1) or np.any(b2)):
        return _kan_numpy(x, w1, b1, w2, b2)

    import ml_dtypes
    from concourse.bass_utils import run_bass_kernel_spmd

    nc = _build_nc()
    bf16 = ml_dtypes.bfloat16

    xT = np.ascontiguousarray(x.T).astype(bf16)          # (DIN, B)
    w1k = (w1 / 64.0).transpose(1, 2, 0).astype(bf16)    # (DIN, K, DOUT)
    w2k = w2.transpose(1, 2, 0).astype(bf16)

    in_maps = []
    for core in range(N_CORES):
        bi, oj = divmod(core, OG)
        osl = slice(oj * OS, (oj + 1) * OS)
        bsl = slice(bi * BS, (bi + 1) * BS)
        # tile-0 rows: [kp, which, k2, OS]; tile-1 rows: [which, K, OS]
        w0 = np.empty((128, 2, 2, 2, OS), dtype=bf16)
        for kp in range(2):
            w0[:, kp, 0] = w1k[0:128, 2 * kp:2 * kp + 2, osl]
            w0[:, kp, 1] = w2k[0:128, 2 * kp:2 * kp + 2, osl]
        w1r = np.empty((128, 2, K, OS), dtype=bf16)
        w1r[:, 0] = w1k[128:256, :, osl]
        w1r[:, 1] = w2k[128:256, :, osl]
        wtc = np.empty((NT * 128, WROW), dtype=bf16)
        wtc[0:128] = w0.reshape(128, WROW)
        wtc[128:256] = w1r.reshape(128, WROW)
        xtc = np.ascontiguousarray(xT[:, bsl]).reshape(NT * 128 * 2, HB)
        in_maps.append({"wt": wtc, "xt": xtc})

    res = run_bass_kernel_spmd(nc, in_maps, core_ids=list(range(N_CORES)))

    out = np.empty((B, DOUT), dtype=np.float32)
    for core in range(N_CORES):
        bi, oj = divmod(core, OG)
        out[bi * BS:(bi + 1) * BS, oj * OS:(oj + 1) * OS] = \
            res.results[core]["outt"].astype(np.float32).T
    return out


# revision 28
# speedup vs baseline: 1.0302x; 1.0089x over previous
"""KAN layer kernel for TRN2, 8-core SPMD.

Math: out[b,o] = sum_{i,k} relu(x[b,i]*w1[o,i,k] + b1[o,i,k]) * w2[o,i,k] / 32 + b2[o]
With b1 == 0 (guaranteed by the generator) the relu is exactly
    relu(x*w1) = (x*w1 + |x|*|w1|) / 2
so the layer collapses to two matmuls with k-pre-reduced weights:
    T[i,o] = sum_k w1*w2        U[i,o] = sum_k |w1|*w2
    out    = x @ T/64 + |x| @ U/64 + b2          (1/64 = 0.5/32)
On device U is formed as (sum_k 2*relu(w1)*w2) - T (relu is the only
ISA-legal rectifier on DVE); both identities are exact for b1 == 0.
The /64 is folded into the staged w1 (exact: a power of two), so the
epilogue is a pure PSUM->bf16 copy and the T-term matmul rhs is the raw
DMA'd x (no min/relu work on x at all; only |x| for the U term).

Sharding: 4 batch groups x 2 dout groups (core = bi*2 + oj); per core
BS=512 batch rows, OS=128 dout cols, 2 din tiles of 128.

Schedule (cost-model ns):  transfers serialize on the DMA engines at
~364ns/128KB; descriptor gen pipelines on SP-HWDGE (625/dma) and Pool
SWDGE (~1040/dma, parallel path).  ACT issues no DMA - an ACT DMA would
steal the 2nd HWDGE slot from SP.  Tile-0's weights arrive as two
k-pair chunks so DVE prep starts at ~3.2us instead of ~3.6:
  SP  : w-t0-kp0 | w-t1 | x-t1h0 | out
  Pool: w-t0-kp1 | x-t0 | x-t1h1 | st0/subU0 (t0 final sums)
  ACT : |x0|, |x1|, epilogue-h0
  DVE : per w chunk: r2 = 2*relu(w1s) (two-scalar tensor_scalar; a
        direct abs is not ISA-encodable on DVE), cat = [w1s,r2] (x)
        w2-broadcast (tensor_tensor, 2x), k-sums as pair-adds giving
        s = [sT, sT+sU]; sU by one subtract; epilogue-h1
  PE  : warm-up (p-state ramp), then T0/U0/T1/U1 accumulating matmuls
        per psum half (asymmetric 320/192 split so the tail half is
        cheap).
"""

import numpy as np

B, DIN, DOUT, K = 2048, 256, 256, 4
N_CORES = 8
BG, OG = 4, 2                      # batch groups x dout groups
BS, OS = B // BG, DOUT // OG       # 512 batch rows, 128 dout cols per core
NT = DIN // 128                    # din tiles
HB = BS // 2                       # batch half (x dma granularity)
HB0, HB1 = 320, 192                # asymmetric psum halves
WROW = 2 * OS * K                  # 1024 bf16 els per (tile, din) row
N_WARM = 45                        # PE warm-up matmuls ([128,128] each)
N_WARM2 = 2                        # mid warm-ups between U0 and T1 matmuls

_CACHE = {}


def _build_nc_raw():
    import concourse.bacc as bacc
    import concourse.tile as tile
    from concourse import mybir

    f32 = mybir.dt.float32
    bf16 = mybir.dt.bfloat16
    AF = mybir.ActivationFunctionType
    OP = mybir.AluOpType
    X = mybir.AxisListType.X

    nc = bacc.Bacc("TRN2", target_bir_lowering=False, debug=False,
                   num_devices=N_CORES, num_swdge_queues=2)
    # t0 rows: [kp, 2(which), 2(k), OS] (k-pair major); t1 rows: [2, K, OS]
    wt = nc.dram_tensor("wt", [NT * 128, WROW], bf16, kind="ExternalInput")
    # x rows: [(t*128+i)*2 + h] -> 256 batch cols (h = batch half)
    xt = nc.dram_tensor("xt", [NT * 128 * 2, HB], bf16, kind="ExternalInput")
    outt = nc.dram_tensor("outt", [OS, BS], bf16, kind="ExternalOutput")

    with tile.TileContext(nc) as tc:
        with (
            tc.tile_pool(name="io", bufs=1) as io,
            tc.tile_pool(name="work", bufs=1) as work,
            tc.tile_pool(name="pp", bufs=1, space="PSUM") as pp,
        ):
            # ---- SBUF tiles.  w chunk tiles have 3 slots: 0=w1 (dma),
            # 1=|w1| (computed), 2=w2 (dma); the cat-mul reads [:, 0::2].
            wa = io.tile([128, 3, 2, OS], bf16, name="wa", tag="wa")
            wb = io.tile([128, 3, 2, OS], bf16, name="wb", tag="wb")
            w1t = io.tile([128, 3, K, OS], bf16, name="w1t", tag="w1t")
            x_sb = [io.tile([128, BS], bf16, name=f"x{t}", tag=f"x{t}")
                    for t in range(NT)]
            xa_sb = [io.tile([128, BS], bf16, name=f"xa{t}", tag=f"xa{t}")
                     for t in range(NT)]
            out_sb = work.tile([128, BS], bf16, tag="outsb")
            zt = work.tile([128, 128], bf16, tag="zt")

            # ---- input DMAs
            # SP (HWDGE): w-t0-kp0, w-t1, x-t1h0
            nc.sync.dma_start(out=wa[:, 0::2], in_=wt[0:128, 0:512])
            nc.sync.dma_start(out=w1t[:, 0::2], in_=wt[128:256, :])
            nc.sync.dma_start(out=x_sb[1][:, 0:HB], in_=xt[256:512:2, :])
            # Pool (SWDGE): w-t0-kp1, x-t0, x-t1h1
            nc.gpsimd.dma_start(out=wb[:, 0::2], in_=wt[0:128, 512:1024])
            nc.gpsimd.dma_start(out=x_sb[0], in_=xt[0:256, :])
            nc.gpsimd.dma_start(out=x_sb[1][:, HB:BS], in_=xt[257:512:2, :])

            # ---- PE warm-up (p-state ramp; matmuls hit 2.4GHz only after
            # ~3us of PE activity in the cost model)
            nc.vector.memset(zt, 0.0)
            pz = pp.tile([128, 128], f32, tag="pz")
            for _ in range(N_WARM):
                nc.tensor.matmul(pz, lhsT=zt, rhs=zt, start=True, stop=True)

            # ---- DVE weight prep
            # per chunk: |w1| -> cat=[w1,|w1|]*w2 -> pair-sum over k
            s0 = work.tile([128, 2, 2, OS], bf16, name="s0", tag="s0")
            st0 = work.tile([128, 2, OS], bf16, name="st0", tag="st0")
            st1 = work.tile([128, 2, OS], bf16, name="st1", tag="st1")
            su0 = work.tile([128, OS], bf16, name="su0", tag="su0")
            su1 = work.tile([128, OS], bf16, name="su1", tag="su1")
            # wait_until hints (~data-landing times) keep the list scheduler
            # from hoisting chunk-b / tile-1 ops in front of ready chunk-a
            # work, which would idle DVE.
            with nc.allow_low_precision(reason="bf16 k-sum, tol 2e-2"):
                for c, wc in enumerate((wa, wb)):
                    cat = work.tile([128, 2, 2, OS], bf16, name=f"cat{c}",
                                    tag=f"cat{c}")
                    with tc.tile_wait_until(0.0 if c == 0 else 0.0037):
                        nc.vector.tensor_scalar(out=wc[:, 1], in0=wc[:, 0],
                                                scalar1=0.0, scalar2=2.0,
                                                op0=OP.max, op1=OP.mult)
                        nc.vector.tensor_tensor(
                            out=cat, in0=wc[:, 0:2],
                            in1=wc[:, 2].unsqueeze(1).broadcast_to(
                                [128, 2, 2, OS]),
                            op=OP.mult)
                        nc.vector.tensor_tensor(out=s0[:, c],
                                                in0=cat[:, :, 0],
                                                in1=cat[:, :, 1], op=OP.add)
                # st0 + subU0 on Pool: frees the serial DVE chain; Pool
                # is idle once its DMA descriptor-gens are done.
                nc.gpsimd.tensor_tensor(out=st0, in0=s0[:, 0], in1=s0[:, 1],
                                        op=OP.add)
                nc.gpsimd.tensor_tensor(out=su0, in0=st0[:, 1],
                                        in1=st0[:, 0], op=OP.subtract)
                # tile 1 in one piece: abs, cat-mul, k pair-sums, final sum
                cat1 = work.tile([128, 2, K, OS], bf16, name="cat1", tag="cat1")
                s21 = work.tile([128, 2, 2, OS], bf16, name="s21", tag="s21")
                with tc.tile_wait_until(0.0042):
                    nc.vector.tensor_scalar(out=w1t[:, 1], in0=w1t[:, 0],
                                            scalar1=0.0, scalar2=2.0,
                                            op0=OP.max, op1=OP.mult)
                    nc.vector.tensor_tensor(
                        out=cat1, in0=w1t[:, 0:2],
                        in1=w1t[:, 2].unsqueeze(1).broadcast_to(
                            [128, 2, K, OS]),
                        op=OP.mult)
                    nc.vector.tensor_tensor(out=s21, in0=cat1[:, :, 0:2],
                                            in1=cat1[:, :, 2:4], op=OP.add)
                    nc.vector.tensor_tensor(out=st1, in0=s21[:, :, 0],
                                            in1=s21[:, :, 1], op=OP.add)
                    nc.vector.tensor_tensor(out=su1, in0=st1[:, 1],
                                            in1=st1[:, 0], op=OP.subtract)

            # ---- |x| on ACT (x1 split so the h0 strip lands first)
            nc.scalar.activation(xa_sb[0], x_sb[0], AF.Abs)
            nc.scalar.activation(xa_sb[1][:, 0:HB0], x_sb[1][:, 0:HB0], AF.Abs)
            nc.scalar.activation(xa_sb[1][:, HB0:BS], x_sb[1][:, HB0:BS],
                                 AF.Abs)

            # ---- matmuls: psum halves = batch slices [0:320) [320:512)
            s_t = (st0, st1)
            psum = [pp.tile([128, hb], f32, name=f"ps{h}", tag=f"ps{h}")
                    for h, hb in enumerate((HB0, HB1))]
            sl = (slice(0, HB0), slice(HB0, BS))
            MM = [
                (0, 0, 0, True, False), (0, 0, 1, True, False),   # T0 h0,h1
                (0, 1, 0, False, False), (0, 1, 1, False, False), # U0
                (1, 0, 0, False, False),                          # T1 h0
                (1, 1, 0, False, True),                           # U1 h0 stop
                (1, 0, 1, False, False),                          # T1 h1
                (1, 1, 1, False, True),                           # U1 h1 stop
            ]
            su = (su0, su1)
            for i, (t, j, h, st, sp) in enumerate(MM):
                if i == 4:
                    for _ in range(N_WARM2):
                        nc.tensor.matmul(pz, lhsT=zt, rhs=zt,
                                         start=True, stop=True)
                rhs = (x_sb if j == 0 else xa_sb)[t][:, sl[h]]
                lhsT = s_t[t][:, 0] if j == 0 else su[t]
                nc.tensor.matmul(psum[h], lhsT=lhsT, rhs=rhs,
                                 start=st, stop=sp)

            # ---- epilogue: pure copy psum -> bf16 (scale folded into w1s)
            nc.scalar.activation(out_sb[:, 0:HB0], psum[0], AF.Identity)
            nc.vector.tensor_copy(out_sb[:, HB0:BS], psum[1])

# ---- output
            nc.sync.dma_start(out=outt[:, :], in_=out_sb)

    nc.compile()
    return nc


def _build_nc():
    if "nc" in _CACHE:
        return _CACHE["nc"]
    nc = _build_nc_raw()
    _CACHE["nc"] = nc
    return nc


def _kan_numpy(x, w1, b1, w2, b2):
    out = np.empty((x.shape[0], w1.shape[0]), dtype=np.float32)
    d = (w1.shape[0] + w1.shape[1]) / 2
    s = 1.0 / np.sqrt(d * w1.shape[2])
    for lo in range(0, x.shape[0], 128):
        hi = min(lo + 128, x.shape[0])
        h = x[lo:hi, None, :, None] * w1[None] + b1[None]
        np.maximum(h, 0.0, out=h)
        out[lo:hi] = np.einsum("boik,oik->bo", h, w2) * s
    return out + b2[None, :]


def kernel(x, w1, b1, w2, b2):
    x = np.asarray(x, dtype=np.float32)
    w1 = np.asarray(w1, dtype=np.float32)
    b1 = np.asarray(b1, dtype=np.float32)
    w2 = np.asarray(w2, dtype=np.float32)
    b2 = np.asarray(b2, dtype=np.float32)

    if (x.shape != (B, DIN) or w1.shape != (DOUT, DIN, K)
            or np.any(b1) or np.any(b2)):
        return _kan_numpy(x, w1, b1, w2, b2)

    import ml_dtypes
    from concourse.bass_utils import run_bass_kernel_spmd

    nc = _build_nc()
    bf16 = ml_dtypes.bfloat16

    xT = np.ascontiguousarray(x.T).astype(bf16)          # (DIN, B)
    w1k = (w1 / 64.0).transpose(1, 2, 0).astype(bf16)    # (DIN, K, DOUT)
    w2k = w2.transpose(1, 2, 0).astype(bf16)

    in_maps = []
    for core in range(N_CORES):
        bi, oj = divmod(core, OG)
        osl = slice(oj * OS, (oj + 1) * OS)
        bsl = slice(bi * BS, (bi + 1) * BS)
        # tile-0 rows: [kp, which, k2, OS]; tile-1 rows: [which, K, OS]
        w0 = np.empty((128, 2, 2, 2, OS), dtype=bf16)
        for kp in range(2):
            w0[:, kp, 0] = w1k[0:128, 2 * kp:2 * kp + 2, osl]
            w0[:, kp, 1] = w2k[0:128, 2 * kp:2 * kp + 2, osl]
        w1r = np.empty((128, 2, K, OS), dtype=bf16)
        w1r[:, 0] = w1k[128:256, :, osl]
        w1r[:, 1] = w2k[128:256, :, osl]
        wtc = np.empty((NT * 128, WROW), dtype=bf16)
        wtc[0:128] = w0.reshape(128, WROW)
        wtc[128:256] = w1r.reshape(128, WROW)
        xtc = np.ascontiguousarray(xT[:, bsl]).reshape(NT * 128 * 2, HB)
        in_maps.append({"wt": wtc, "xt": xtc})

    res = run_bass_kernel_spmd(nc, in_maps, core_ids=list(range(N_CORES)))

    out = np.empty((B, DOUT), dtype=np.float32)
    for core in range(N_CORES):
        bi, oj = divmod(core, OG)
        out[bi * BS:(bi + 1) * BS, oj * OS:(oj + 1) * OS] = \
            res.results[core]["outt"].astype(np.float32).T
    return out
